# revision 46
# baseline (speedup 1.0000x reference)
"""WLS log-linear DTI FA kernel for 8 Trainium2 NeuronCores.

Reference computation (per voxel v of a 100^3 volume, 64 gradient dirs):
    s      = ln(max(dwi[v], min_diffusivity))          [64]
    fit    = design_matrix_inv[:6] @ s                 [6]
    T      = sym3x3(fit) (+ tiny SymEig noise)
    eig    = eigenvalues(T) clamped to min_diffusivity
    fa[v]  = sqrt(0.5*sum (ei-ej)^2 / sum ei^2) * mask[v]

Kernel strategy (data-parallel over voxels, 8 cores):
  - FA is computed from tensor INVARIANTS instead of eigenvalues:
        FA = sqrt(1.5 * p2 / (p2 + tr^2/3))
    with p2 = ||deviatoric(T)||_F^2 and tr = trace(T). This matches the
    eigen-clamped reference to ~5e-3 rel-L2 on this input distribution
    (only ~0.4% of voxels clamp an eigenvalue; SymEig noise contributes
    ~1e-3) - validated against the jax reference on the full volume.
  - host pre-transposes dwi to the device layout (partition = 64*u+g,
    i.e. grads on partitions, two voxels per 128-partition column), so the
    device does a straight wide DMA (no DMA-transpose, no xbar descriptors).
  - per tile: ACT Ln (clamp folded into the activation bias) -> one
    [128x128] x [128x14] matmul per 128 pair-columns. The 14 moving
    columns hold per-voxel-half [aa, d, bb, e, f, cc, tr]: deviatoric
    components and trace as LINEAR combinations folded into the weights.
  - a single-input custom DVE op squares the PSUM tile (FA needs only the
    squares of the 7 outputs, and this pass doubles as the PSUM evac). An
    all-DVE per-group chain (no cross-engine ping-pong: in-order queues
    convoy otherwise) reduces to p2/den and computes fa = sqrt(1.5*p2/den)
    with a fused reciprocal (bit-trick + 2 Newton steps in ONE pass) and a
    fused Newton sqrt seeded by the fast-inverse-sqrt magic constant
    evaluated in the float domain (DVE has no integer shifts).
  - ACT runs nothing but the big Ln passes: a single act-table load, no
    trig, no table thrash. FA groups shrink toward the end (the last tile
    is a 3-chunk runt) so only one short chain trails the last Ln.
Host: shard/pad/pack dwi, gather/unpermute FA, multiply by mask.
"""
import sys
import types

import numpy as np
import ml_dtypes

import concourse.bass as bass
import concourse.mybir as mybir
import concourse.tile as tile
from concourse import bacc, bass_utils


def _ensure_ntff_hook():
    """bass_utils imports antenv.axon_hooks when tracing; some images lack it.
    Register a shim backed by the axon .so so NTFF profiling works (or a no-op
    getter so runs degrade to trace-less instead of crashing)."""
    try:
        import antenv.axon_hooks  # noqa: F401
        return
    except ImportError:
        pass
    try:
        from trn_agent_boot.trn_boot import _ntff_profile_via_ctypes

        hook = _ntff_profile_via_ctypes("/opt/axon/libaxon_pjrt.so")
    except Exception:
        hook = None
    mod = types.ModuleType("antenv.axon_hooks")
    mod._hook = hook
    mod.get_axon_ntff_profile_hook = lambda: mod._hook
    mod.set_axon_ntff_profile_hook = lambda h: setattr(mod, "_hook", h)
    sys.modules["antenv.axon_hooks"] = mod
    try:
        import antenv

        antenv.axon_hooks = mod
    except ImportError:
        pass


_ensure_ntff_hook()

AFT = mybir.ActivationFunctionType
ALU = mybir.AluOpType
BF16_NP = np.dtype(ml_dtypes.bfloat16)

# ---- fixed problem geometry (hardcoded per contract) ----
NX = NY = NZ = 100
V_TOT = NX * NY * NZ            # 1,000,000 voxels
G = 64                          # gradient directions
NCORES = 8
V_CORE = V_TOT // NCORES        # 125,000 voxels per core

P = 128                         # SBUF partitions
NCH = 489                       # global matmul chunks of 128 pair-columns
NPAIR = NCH * 128               # 62,592 voxel-pair columns
V_PAD = NPAIR * 2               # 125,184 >= V_CORE (0.15% padding)
F_FA = NCH * 2                  # voxels per partition in FA phase
K = 14                          # matmul outputs per pair-column (7 per voxel)
# tile widths in chunks: uniform stream + one small runt at the end so the
# post-last-Ln tail (matmul+squares+FA chain of the final tile) is short
CH_TILES = [27] * 18 + [3]
assert sum(CH_TILES) == NCH
T_TILES = len(CH_TILES)
CH_LO = [sum(CH_TILES[:i]) for i in range(T_TILES)]  # chunk offset per tile

SQRT_MAGIC = 0x5F3759DF         # fast inverse-sqrt seed constant


# ------------------------------------------------------------------
# custom DVE ops (registered into concourse.dve_ops at import time)
# ------------------------------------------------------------------
def _register_dve_ops():
    """Register the fused FA ops. Single-pass DVE instructions:
      FA_DEN : out = max(Src0 + Src1*C0, C1)            (den = p2 + tr2/3)
      FA_NR1 : out = Src1*(C0 - Src0*sq(Src1)*C1)       (rsqrt Newton step)
      FA_NR2M: out = Src0*Src1*(C0 - Src0*sq(Src1)*C1)  (step + mult by x)
    uops_sha is computed here (self-consistent pin) rather than hardcoded."""
    if "dve_ops" in _cache:
        return _cache["dve_ops"]
    import concourse.dve_ops as dve_ops
    from concourse.dve_ops import DveOp, OPS, CUSTOM_DVE_SPECS, _SUB_OPCODE_FOR_NAME
    from concourse.dve_spec import Spec, Src0, Src1, C0, C1, sq, maxx, lower
    from concourse.dve_spec import _has_src1 as has_src1
    from concourse.dve_uop import DveOpSpec

    defs = [
        (
            "FA_SQ",
            Spec(
                body=sq(Src0),
                reference=lambda in0, in1, c0, c1, c2: in0.astype(np.float32) ** 2,
            ),
        ),
        (
            "FA_DEN",
            Spec(
                body=maxx(Src0 + Src1 * C0, C1),
                reference=lambda in0, in1, c0, c1, c2: np.maximum(
                    in0.astype(np.float32) + in1 * c0, c1
                ),
            ),
        ),
        (
            "FA_NR1",
            Spec(
                body=Src1 * (C0 - Src0 * sq(Src1) * C1),
                reference=lambda in0, in1, c0, c1, c2: in1
                * (c0 - in0.astype(np.float32) * in1 * in1 * c1),
            ),
        ),
        (
            "FA_NR2M",
            Spec(
                body=(Src0 * Src1) * (C0 - Src0 * sq(Src1) * C1),
                reference=lambda in0, in1, c0, c1, c2: in0.astype(np.float32)
                * in1
                * (c0 - in0 * in1 * in1 * c1),
            ),
        ),
    ]
    handles = {}
    for name, spec in defs:
        if name in _SUB_OPCODE_FOR_NAME:
            handles[name] = next(o for o in OPS if o.name == name)
            continue
        row = max(_SUB_OPCODE_FOR_NAME.values()) + 1
        assert row < 0x20, "custom DVE opcode rows exhausted"
        _SUB_OPCODE_FOR_NAME[name] = row
        shas = {}
        for ver in ("v3", "v4"):
            s = DveOpSpec(
                name=name, opcode=row, uops=lower(spec, ver=ver),
                rd1_en=has_src1(spec),
            ).sha(ver)
            shas[ver] = s
        op = DveOp(name, spec, subdim=False, uops_sha=shas)
        OPS.append(op)
        CUSTOM_DVE_SPECS[name] = spec
        handles[name] = op
    _cache["dve_ops"] = handles
    return handles


# ------------------------------------------------------------------
# host-side helpers
# ------------------------------------------------------------------
_cache = {}


def _voxel_map():
    """vmap[p, gch, u] = padded-shard voxel index at device position
    (partition p of chunk gch, pair-half u): v = 256*gch + 2*p + u."""
    if "vmap" in _cache:
        return _cache["vmap"]
    p = np.arange(P)[:, None, None]
    gch = np.arange(NCH)[None, :, None]
    u = np.arange(2)[None, None, :]
    vmap = 256 * gch + 2 * p + u  # [P, NCH, 2]
    _cache["vmap"] = vmap
    return vmap


def _wpat(design_matrix_inv):
    """Block-diagonal weight pattern [128, 14] bf16:
    wpat[64*u+g, 7*u+m] = wp7[m, g] with rows [aa, d, bb, e, f, cc, tr]:
    deviatoric diag combos and the trace, all linear in s."""
    w6 = np.asarray(design_matrix_inv, dtype=np.float32)[:6]  # a d b e f c
    wtr = w6[0] + w6[2] + w6[5]
    wp7 = np.stack(
        [w6[0] - wtr / 3, w6[1], w6[2] - wtr / 3, w6[3], w6[4], w6[5] - wtr / 3, wtr]
    ).astype(np.float32)  # [7, 64]
    wpat = np.zeros((P, K), dtype=np.float32)
    for u in range(2):
        wpat[64 * u : 64 * u + 64, 7 * u : 7 * u + 7] = wp7.T
    return np.ascontiguousarray(wpat.astype(BF16_NP))


def _pack_core(shard_bf16_u16):
    """[V_PAD, 64] uint16 view of bf16 -> flat device layout [128, NPAIR]:
    dwiT[64*u+g, 128*gch+i] = dwi[(gch*128+i)*2+u, g]."""
    a = shard_bf16_u16.reshape(NCH, 128, 2, G)
    a = a.transpose(2, 3, 0, 1)  # [u, g, gch, i]
    return np.ascontiguousarray(a.reshape(P, NPAIR))


# ------------------------------------------------------------------
# device program
# ------------------------------------------------------------------
def _fa_group(nc, fat, sq_all, fa_all, fa_d, ops, lo, hi):
    """FA for chunk range [lo, hi): squares -> p2/den -> fa.
    All inputs are squares of [aa, d, bb, e, f, cc, tr] at stride 7."""
    F = (hi - lo) * 2
    f32 = mybir.dt.float32
    i32 = mybir.dt.int32
    sq = sq_all[:, lo * K : hi * K].rearrange("p (n k) -> p n k", k=7)
    q_aa, q_d, q_bb, q_e, q_f, q_cc, q_tr = (sq[:, :, j] for j in range(7))

    def tl(tag, dt=f32):
        return fat.tile([P, F], dt, tag=tag, name=tag)

    t1 = tl("t1"); t2 = tl("t2"); p2 = tl("p2")
    den = tl("den"); z2 = tl("z2"); y1 = tl("y1")
    sw = tl("sw", i32); sf = tl("sf")

    # the whole chain stays on DVE: in-engine dependencies execute
    # back-to-back with no semaphores, so the per-group latency is short
    # (cross-engine ping-pong head-blocks the in-order queues)
    v = nc.vector
    v.tensor_tensor(out=t1, in0=q_aa, in1=q_bb, op=ALU.add)
    v.tensor_tensor(out=t2, in0=q_d, in1=q_e, op=ALU.add)
    v.tensor_tensor(out=t1, in0=t1, in1=q_cc, op=ALU.add)
    v.tensor_tensor(out=t2, in0=t2, in1=q_f, op=ALU.add)
    # p2 = t1 + 2*t2
    v.scalar_tensor_tensor(out=p2, in0=t2, scalar=2.0, in1=t1, op0=ALU.mult, op1=ALU.add)
    # den = max(p2 + tr2/3, 1e-30)
    v._custom_dve(ops["FA_DEN"], out=den, in0=p2, in1=q_tr, s0=1.0 / 3.0, s1=1e-30)
    # irec = 1/den (bit-trick + 2 Newton, one pass); z2 = fa^2 = 1.5*p2*irec
    v.reciprocal_approx_fast(out=den, in_=den)
    v.scalar_tensor_tensor(out=z2, in0=p2, scalar=1.5, in1=den, op0=ALU.mult, op1=ALU.mult)
    # sqrt(z2): magic rsqrt seed computed in the FLOAT domain (DVE has no
    # shifts): seed_int = round(MAGIC - bits(z2)/2), <=128-ulp off the
    # classic (MAGIC - (bits>>1)) - irrelevant against the 3.4% seed error.
    v.tensor_copy(out=sf, in_=z2.bitcast(i32))          # int -> float value
    v.tensor_scalar(
        out=sf, in0=sf, scalar1=-0.5, scalar2=float(SQRT_MAGIC),
        op0=ALU.mult, op1=ALU.add,
    )
    v.tensor_copy(out=sw, in_=sf)                       # float -> int value
    v._custom_dve(ops["FA_NR1"], out=y1, in0=z2, in1=sw.bitcast(f32), s0=1.5, s1=0.5)
    v._custom_dve(
        ops["FA_NR2M"], out=fa_all[:, lo * 2 : hi * 2], in0=z2, in1=y1, s0=1.5, s1=0.5
    )
    # stream this group's FA out now; only the last group's DMA is tail
    nc.sync.dma_start(
        out=fa_d[:, lo * 2 : hi * 2], in_=fa_all[:, lo * 2 : hi * 2]
    )


def _build_program(mind: float):
    ops = _register_dve_ops()
    nc = bacc.Bacc("TRN2", target_bir_lowering=False, debug=False, num_devices=NCORES)
    f32 = mybir.dt.float32
    bf16 = mybir.dt.bfloat16

    dwi_d = nc.dram_tensor("dwi", [P, NPAIR], bf16, kind="ExternalInput")
    wpat_d = nc.dram_tensor("wpat", [P, K], bf16, kind="ExternalInput")
    fa_d = nc.dram_tensor("fa", [P, F_FA], f32, kind="ExternalOutput")

    # FA group boundaries (in tiles): big early, progressively smaller so
    # each chain overlaps the remaining Ln stream; the last is the runt
    SLICES = [(0, 4), (4, 8), (8, 12), (12, 15), (15, 17), (17, 18), (18, 19)]
    slice_end = {hi: (lo, hi) for lo, hi in SLICES}
    # tile 0 is DMA'd and Ln'd in three chunks (on separate queues) so the
    # first Ln starts as soon as the first few hundred KB land. Spans stay
    # >=512 cols: small activations pick a different act-table variant and
    # would trigger a second 1.28us table load.
    W0 = CH_TILES[0] * 128
    T0_SPANS = [(0, 512), (512, 1536), (1536, W0)]

    with tile.TileContext(nc) as tc:
        with (
            tc.tile_pool(name="singles", bufs=1) as singles,
            tc.tile_pool(name="persist", bufs=1) as persist,
            tc.tile_pool(name="tsp", bufs=10) as tsp_pool,
            tc.tile_pool(name="psum", bufs=8, space="PSUM") as psum_pool,
            tc.tile_pool(name="fat", bufs=2) as fat,
        ):
            consts = singles.tile([P, 2], f32, tag="consts", name="consts")
            nc.vector.memset(consts[:, 0:1], mind)

            sq_all = persist.tile([P, NCH * K], f32, tag="sq", name="sq_all")
            fa_all = persist.tile([P, F_FA], f32, tag="fa", name="fa_all")
            wpat_sb = singles.tile([P, K], bf16, tag="wpat", name="wpat_sb")

            # all input DMA on the sync hwdge queue: each hardware DMA
            # queue has a multi-us first-use penalty, so one warm queue
            # beats spreading across cold ones
            for t in range(T_TILES):
                wt = CH_TILES[t] * 128
                sT = tsp_pool.tile([P, wt], bf16, tag="sT", name="sT")
                spans = T0_SPANS if t == 0 else [(0, wt)]
                for si, (a, b) in enumerate(spans):
                    eng = nc.sync
                    eng.dma_start(
                        out=sT[:, a:b],
                        in_=dwi_d[:, CH_LO[t] * 128 + a : CH_LO[t] * 128 + b],
                    )
                    # s = ln(dwi + mind)  (~= ln(max(dwi, mind)); dwi >= 0)
                    nc.scalar.activation(
                        out=sT[:, a:b], in_=sT[:, a:b], func=AFT.Ln,
                        bias=consts[:, 0:1],
                    )
                if t == 0:
                    nc.gpsimd.dma_start(out=wpat_sb, in_=wpat_d[:, :])
                pt = psum_pool.tile([P, CH_TILES[t] * K], f32, tag="ps", name="ps")
                for c in range(CH_TILES[t]):
                    nc.tensor.matmul(
                        out=pt[:, c * K : (c + 1) * K],
                        lhsT=sT[:, c * 128 : (c + 1) * 128],
                        rhs=wpat_sb,
                        start=True,
                        stop=True,
                    )
                # squares are all FA needs; also the PSUM evac (GPSIMD
                # cannot touch PSUM; one PSUM read per DVE instruction)
                nc.vector._custom_dve(
                    ops["FA_SQ"],
                    out=sq_all[:, CH_LO[t] * K : (CH_LO[t] + CH_TILES[t]) * K],
                    in0=pt,
                )
                if (t + 1) in slice_end:
                    lo_t, hi_t = slice_end[t + 1]
                    _fa_group(nc, fat, sq_all, fa_all, fa_d, ops,
                              CH_LO[lo_t],
                              CH_LO[hi_t] if hi_t < T_TILES else NCH)

    nc.compile()
    return nc


def _get_program(mind: float):
    key = ("prog", round(mind, 18))
    if key not in _cache:
        _cache[key] = _build_program(mind)
    return _cache[key]


# ------------------------------------------------------------------
# entry point
# ------------------------------------------------------------------
def kernel(dwi, mask, design_matrix_inv, min_diffusivity):
    dwi = np.ascontiguousarray(np.asarray(dwi, dtype=np.float32)).reshape(V_TOT, G)
    mask = np.asarray(mask, dtype=np.float32).reshape(V_TOT)
    mind = float(np.asarray(min_diffusivity))

    nc = _get_program(mind)
    wpat = _wpat(design_matrix_inv)

    dwi_bf = dwi.astype(BF16_NP).view(np.uint16)  # [V_TOT, 64] bf16 bits
    in_maps = []
    for core in range(NCORES):
        pad = np.empty((V_PAD, G), dtype=np.uint16)
        pad[:V_CORE] = dwi_bf[core * V_CORE : (core + 1) * V_CORE]
        pad[V_CORE:] = np.float32(1.0).astype(BF16_NP).view(np.uint16)
        in_maps.append(
            {"dwi": _pack_core(pad).view(BF16_NP), "wpat": wpat}
        )

    res = None
    for attempt in range(3):
        try:
            res = bass_utils.run_bass_kernel_spmd(nc, in_maps, core_ids=list(range(NCORES)))
            break
        except Exception:
            if attempt == 2:
                raise
    _cache["last_result"] = res  # exec_time_ns etc. for the dev harness

    vmap_flat = _voxel_map().reshape(-1)  # [P*NCH*2]
    fa = np.empty(V_TOT, dtype=np.float32)
    for core in range(NCORES):
        fa_dev = np.asarray(res.results[core]["fa"]).reshape(-1)
        fa_pad = np.empty(V_PAD, dtype=np.float32)
        fa_pad[vmap_flat] = fa_dev
        fa[core * V_CORE : (core + 1) * V_CORE] = fa_pad[:V_CORE]

    fa *= mask
    return fa.reshape(NX, NY, NZ, 1)


# revision 47
# speedup vs baseline: 1.0014x; 1.0014x over previous
"""WLS log-linear DTI FA kernel for 8 Trainium2 NeuronCores.

Reference computation (per voxel v of a 100^3 volume, 64 gradient dirs):
    s      = ln(max(dwi[v], min_diffusivity))          [64]
    fit    = design_matrix_inv[:6] @ s                 [6]
    T      = sym3x3(fit) (+ tiny SymEig noise)
    eig    = eigenvalues(T) clamped to min_diffusivity
    fa[v]  = sqrt(0.5*sum (ei-ej)^2 / sum ei^2) * mask[v]

Kernel strategy (data-parallel over voxels, 8 cores):
  - FA is computed from tensor INVARIANTS instead of eigenvalues:
        FA = sqrt(1.5 * p2 / (p2 + tr^2/3))
    with p2 = ||deviatoric(T)||_F^2 and tr = trace(T). This matches the
    eigen-clamped reference to ~5e-3 rel-L2 on this input distribution
    (only ~0.4% of voxels clamp an eigenvalue; SymEig noise contributes
    ~1e-3) - validated against the jax reference on the full volume.
  - host pre-transposes dwi to the device layout (partition = 64*u+g,
    i.e. grads on partitions, two voxels per 128-partition column), so the
    device does a straight wide DMA (no DMA-transpose, no xbar descriptors).
  - per tile: ACT Ln (clamp folded into the activation bias) -> one
    [128x128] x [128x14] matmul per 128 pair-columns. The 14 moving
    columns hold per-voxel-half [aa, d, bb, e, f, cc, tr]: deviatoric
    components and trace as LINEAR combinations folded into the weights.
  - a single-input custom DVE op squares the PSUM tile (FA needs only the
    squares of the 7 outputs, and this pass doubles as the PSUM evac). An
    all-DVE per-group chain (no cross-engine ping-pong: in-order queues
    convoy otherwise) reduces to p2/den and computes fa = sqrt(1.5*p2/den)
    with a fused reciprocal (bit-trick + 2 Newton steps in ONE pass) and a
    fused Newton sqrt seeded by the fast-inverse-sqrt magic constant
    evaluated in the float domain (DVE has no integer shifts).
  - ACT runs nothing but the big Ln passes: a single act-table load, no
    trig, no table thrash. FA groups shrink toward the end (the last tile
    is a 3-chunk runt) so only one short chain trails the last Ln.
Host: shard/pad/pack dwi, gather/unpermute FA, multiply by mask.
"""
import sys
import types

import numpy as np
import ml_dtypes

import concourse.bass as bass
import concourse.mybir as mybir
import concourse.tile as tile
from concourse import bacc, bass_utils


def _ensure_ntff_hook():
    """bass_utils imports antenv.axon_hooks when tracing; some images lack it.
    Register a shim backed by the axon .so so NTFF profiling works (or a no-op
    getter so runs degrade to trace-less instead of crashing)."""
    try:
        import antenv.axon_hooks  # noqa: F401
        return
    except ImportError:
        pass
    try:
        from trn_agent_boot.trn_boot import _ntff_profile_via_ctypes

        hook = _ntff_profile_via_ctypes("/opt/axon/libaxon_pjrt.so")
    except Exception:
        hook = None
    mod = types.ModuleType("antenv.axon_hooks")
    mod._hook = hook
    mod.get_axon_ntff_profile_hook = lambda: mod._hook
    mod.set_axon_ntff_profile_hook = lambda h: setattr(mod, "_hook", h)
    sys.modules["antenv.axon_hooks"] = mod
    try:
        import antenv

        antenv.axon_hooks = mod
    except ImportError:
        pass


_ensure_ntff_hook()

AFT = mybir.ActivationFunctionType
ALU = mybir.AluOpType
BF16_NP = np.dtype(ml_dtypes.bfloat16)

# ---- fixed problem geometry (hardcoded per contract) ----
NX = NY = NZ = 100
V_TOT = NX * NY * NZ            # 1,000,000 voxels
G = 64                          # gradient directions
NCORES = 8
V_CORE = V_TOT // NCORES        # 125,000 voxels per core

P = 128                         # SBUF partitions
NCH = 489                       # global matmul chunks of 128 pair-columns
NPAIR = NCH * 128               # 62,592 voxel-pair columns
V_PAD = NPAIR * 2               # 125,184 >= V_CORE (0.15% padding)
F_FA = NCH * 2                  # voxels per partition in FA phase
K = 14                          # matmul outputs per pair-column (7 per voxel)
# tile widths in chunks: uniform stream + one small runt at the end so the
# post-last-Ln tail (matmul+squares+FA chain of the final tile) is short
CH_TILES = [27] * 18 + [3]
assert sum(CH_TILES) == NCH
T_TILES = len(CH_TILES)
CH_LO = [sum(CH_TILES[:i]) for i in range(T_TILES)]  # chunk offset per tile

SQRT_MAGIC = 0x5F3759DF         # fast inverse-sqrt seed constant


# ------------------------------------------------------------------
# custom DVE ops (registered into concourse.dve_ops at import time)
# ------------------------------------------------------------------
def _register_dve_ops():
    """Register the fused FA ops. Single-pass DVE instructions:
      FA_DEN : out = max(Src0 + Src1*C0, C1)            (den = p2 + tr2/3)
      FA_NR1 : out = Src1*(C0 - Src0*sq(Src1)*C1)       (rsqrt Newton step)
      FA_NR2M: out = Src0*Src1*(C0 - Src0*sq(Src1)*C1)  (step + mult by x)
    uops_sha is computed here (self-consistent pin) rather than hardcoded."""
    if "dve_ops" in _cache:
        return _cache["dve_ops"]
    import concourse.dve_ops as dve_ops
    from concourse.dve_ops import DveOp, OPS, CUSTOM_DVE_SPECS, _SUB_OPCODE_FOR_NAME
    from concourse.dve_spec import Spec, Src0, Src1, C0, C1, sq, maxx, lower
    from concourse.dve_spec import _has_src1 as has_src1
    from concourse.dve_uop import DveOpSpec

    defs = [
        (
            "FA_SQ",
            Spec(
                body=sq(Src0),
                reference=lambda in0, in1, c0, c1, c2: in0.astype(np.float32) ** 2,
            ),
        ),
        (
            "FA_DEN",
            Spec(
                body=maxx(Src0 + Src1 * C0, C1),
                reference=lambda in0, in1, c0, c1, c2: np.maximum(
                    in0.astype(np.float32) + in1 * c0, c1
                ),
            ),
        ),
        (
            "FA_NR1",
            Spec(
                body=Src1 * (C0 - Src0 * sq(Src1) * C1),
                reference=lambda in0, in1, c0, c1, c2: in1
                * (c0 - in0.astype(np.float32) * in1 * in1 * c1),
            ),
        ),
        (
            "FA_NR2M",
            Spec(
                body=(Src0 * Src1) * (C0 - Src0 * sq(Src1) * C1),
                reference=lambda in0, in1, c0, c1, c2: in0.astype(np.float32)
                * in1
                * (c0 - in0 * in1 * in1 * c1),
            ),
        ),
    ]
    handles = {}
    for name, spec in defs:
        if name in _SUB_OPCODE_FOR_NAME:
            handles[name] = next(o for o in OPS if o.name == name)
            continue
        row = max(_SUB_OPCODE_FOR_NAME.values()) + 1
        assert row < 0x20, "custom DVE opcode rows exhausted"
        _SUB_OPCODE_FOR_NAME[name] = row
        shas = {}
        for ver in ("v3", "v4"):
            s = DveOpSpec(
                name=name, opcode=row, uops=lower(spec, ver=ver),
                rd1_en=has_src1(spec),
            ).sha(ver)
            shas[ver] = s
        op = DveOp(name, spec, subdim=False, uops_sha=shas)
        OPS.append(op)
        CUSTOM_DVE_SPECS[name] = spec
        handles[name] = op
    _cache["dve_ops"] = handles
    return handles


# ------------------------------------------------------------------
# host-side helpers
# ------------------------------------------------------------------
_cache = {}


def _voxel_map():
    """vmap[p, gch, u] = padded-shard voxel index at device position
    (partition p of chunk gch, pair-half u): v = 256*gch + 2*p + u."""
    if "vmap" in _cache:
        return _cache["vmap"]
    p = np.arange(P)[:, None, None]
    gch = np.arange(NCH)[None, :, None]
    u = np.arange(2)[None, None, :]
    vmap = 256 * gch + 2 * p + u  # [P, NCH, 2]
    _cache["vmap"] = vmap
    return vmap


def _wpat(design_matrix_inv):
    """Block-diagonal weight pattern [128, 14] bf16:
    wpat[64*u+g, 7*u+m] = wp7[m, g] with rows [aa, d, bb, e, f, cc, tr]:
    deviatoric diag combos and the trace, all linear in s."""
    w6 = np.asarray(design_matrix_inv, dtype=np.float32)[:6]  # a d b e f c
    wtr = w6[0] + w6[2] + w6[5]
    wp7 = np.stack(
        [w6[0] - wtr / 3, w6[1], w6[2] - wtr / 3, w6[3], w6[4], w6[5] - wtr / 3, wtr]
    ).astype(np.float32)  # [7, 64]
    wpat = np.zeros((P, K), dtype=np.float32)
    for u in range(2):
        wpat[64 * u : 64 * u + 64, 7 * u : 7 * u + 7] = wp7.T
    return np.ascontiguousarray(wpat.astype(BF16_NP))


def _pack_core(shard_bf16_u16):
    """[V_PAD, 64] uint16 view of bf16 -> flat device layout [128, NPAIR]:
    dwiT[64*u+g, 128*gch+i] = dwi[(gch*128+i)*2+u, g]."""
    a = shard_bf16_u16.reshape(NCH, 128, 2, G)
    a = a.transpose(2, 3, 0, 1)  # [u, g, gch, i]
    return np.ascontiguousarray(a.reshape(P, NPAIR))


# ------------------------------------------------------------------
# device program
# ------------------------------------------------------------------
def _fa_group(nc, fat, sq_all, fa_all, fa_d, ops, lo, hi):
    """FA for chunk range [lo, hi): squares -> p2/den -> fa.
    All inputs are squares of [aa, d, bb, e, f, cc, tr] at stride 7."""
    F = (hi - lo) * 2
    f32 = mybir.dt.float32
    i32 = mybir.dt.int32
    sq = sq_all[:, lo * K : hi * K].rearrange("p (n k) -> p n k", k=7)
    q_aa, q_d, q_bb, q_e, q_f, q_cc, q_tr = (sq[:, :, j] for j in range(7))

    def tl(tag, dt=f32):
        return fat.tile([P, F], dt, tag=tag, name=tag)

    t1 = tl("t1"); t2 = tl("t2"); p2 = tl("p2")
    den = tl("den"); z2 = tl("z2"); y1 = tl("y1")
    sw = tl("sw", i32); sf = tl("sf")

    # the whole chain stays on DVE: in-engine dependencies execute
    # back-to-back with no semaphores, so the per-group latency is short
    # (cross-engine ping-pong head-blocks the in-order queues)
    v = nc.vector
    v.tensor_tensor(out=t1, in0=q_aa, in1=q_bb, op=ALU.add)
    v.tensor_tensor(out=t2, in0=q_d, in1=q_e, op=ALU.add)
    v.tensor_tensor(out=t1, in0=t1, in1=q_cc, op=ALU.add)
    v.tensor_tensor(out=t2, in0=t2, in1=q_f, op=ALU.add)
    # p2 = t1 + 2*t2
    v.scalar_tensor_tensor(out=p2, in0=t2, scalar=2.0, in1=t1, op0=ALU.mult, op1=ALU.add)
    # den = max(p2 + tr2/3, 1e-30)
    v._custom_dve(ops["FA_DEN"], out=den, in0=p2, in1=q_tr, s0=1.0 / 3.0, s1=1e-30)
    # irec = 1/den (bit-trick + 2 Newton, one pass); z2 = fa^2 = 1.5*p2*irec
    v.reciprocal_approx_fast(out=den, in_=den)
    v.scalar_tensor_tensor(out=z2, in0=p2, scalar=1.5, in1=den, op0=ALU.mult, op1=ALU.mult)
    # sqrt(z2): magic rsqrt seed computed in the FLOAT domain (DVE has no
    # shifts): seed_int = round(MAGIC - bits(z2)/2), <=128-ulp off the
    # classic (MAGIC - (bits>>1)) - irrelevant against the 3.4% seed error.
    v.tensor_copy(out=sf, in_=z2.bitcast(i32))          # int -> float value
    v.tensor_scalar(
        out=sf, in0=sf, scalar1=-0.5, scalar2=float(SQRT_MAGIC),
        op0=ALU.mult, op1=ALU.add,
    )
    v.tensor_copy(out=sw, in_=sf)                       # float -> int value
    v._custom_dve(ops["FA_NR1"], out=y1, in0=z2, in1=sw.bitcast(f32), s0=1.5, s1=0.5)
    v._custom_dve(
        ops["FA_NR2M"], out=fa_all[:, lo * 2 : hi * 2], in0=z2, in1=y1, s0=1.5, s1=0.5
    )
    # stream this group's FA out now; only the last group's DMA is tail
    nc.sync.dma_start(
        out=fa_d[:, lo * 2 : hi * 2], in_=fa_all[:, lo * 2 : hi * 2]
    )


def _build_program(mind: float):
    ops = _register_dve_ops()
    nc = bacc.Bacc("TRN2", target_bir_lowering=False, debug=False, num_devices=NCORES)
    f32 = mybir.dt.float32
    bf16 = mybir.dt.bfloat16

    dwi_d = nc.dram_tensor("dwi", [P, NPAIR], bf16, kind="ExternalInput")
    wpat_d = nc.dram_tensor("wpat", [P, K], bf16, kind="ExternalInput")
    fa_d = nc.dram_tensor("fa", [P, F_FA], f32, kind="ExternalOutput")

    # FA group boundaries (in tiles): big early, progressively smaller so
    # each chain overlaps the remaining Ln stream; the last is the runt
    SLICES = [(0, 4), (4, 8), (8, 12), (12, 15), (15, 17), (17, 18), (18, 19)]
    slice_end = {hi: (lo, hi) for lo, hi in SLICES}
    # tile 0 is DMA'd and Ln'd in three chunks (on separate queues) so the
    # first Ln starts as soon as the first few hundred KB land. Spans stay
    # >=512 cols: small activations pick a different act-table variant and
    # would trigger a second 1.28us table load.
    W0 = CH_TILES[0] * 128
    T0_SPANS = [(0, 512), (512, 1536), (1536, W0)]

    with tile.TileContext(nc) as tc:
        with (
            tc.tile_pool(name="singles", bufs=1) as singles,
            tc.tile_pool(name="persist", bufs=1) as persist,
            tc.tile_pool(name="tsp", bufs=8) as tsp_pool,
            tc.tile_pool(name="psum", bufs=8, space="PSUM") as psum_pool,
            tc.tile_pool(name="fat", bufs=2) as fat,
        ):
            consts = singles.tile([P, 2], f32, tag="consts", name="consts")
            nc.vector.memset(consts[:, 0:1], mind)

            sq_all = persist.tile([P, NCH * K], f32, tag="sq", name="sq_all")
            fa_all = persist.tile([P, F_FA], f32, tag="fa", name="fa_all")
            wpat_sb = singles.tile([P, K], bf16, tag="wpat", name="wpat_sb")

            # all input DMA on the sync hwdge queue: each hardware DMA
            # queue has a multi-us first-use penalty, so one warm queue
            # beats spreading across cold ones
            for t in range(T_TILES):
                wt = CH_TILES[t] * 128
                sT = tsp_pool.tile([P, wt], bf16, tag="sT", name="sT")
                spans = T0_SPANS if t == 0 else [(0, wt)]
                for si, (a, b) in enumerate(spans):
                    eng = nc.sync
                    eng.dma_start(
                        out=sT[:, a:b],
                        in_=dwi_d[:, CH_LO[t] * 128 + a : CH_LO[t] * 128 + b],
                    )
                    # s = ln(dwi + mind)  (~= ln(max(dwi, mind)); dwi >= 0)
                    nc.scalar.activation(
                        out=sT[:, a:b], in_=sT[:, a:b], func=AFT.Ln,
                        bias=consts[:, 0:1],
                    )
                if t == 0:
                    nc.gpsimd.dma_start(out=wpat_sb, in_=wpat_d[:, :])
                pt = psum_pool.tile([P, CH_TILES[t] * K], f32, tag="ps", name="ps")
                for c in range(CH_TILES[t]):
                    nc.tensor.matmul(
                        out=pt[:, c * K : (c + 1) * K],
                        lhsT=sT[:, c * 128 : (c + 1) * 128],
                        rhs=wpat_sb,
                        start=True,
                        stop=True,
                    )
                # squares are all FA needs; also the PSUM evac (GPSIMD
                # cannot touch PSUM; one PSUM read per DVE instruction)
                nc.vector._custom_dve(
                    ops["FA_SQ"],
                    out=sq_all[:, CH_LO[t] * K : (CH_LO[t] + CH_TILES[t]) * K],
                    in0=pt,
                )
                if (t + 1) in slice_end:
                    lo_t, hi_t = slice_end[t + 1]
                    _fa_group(nc, fat, sq_all, fa_all, fa_d, ops,
                              CH_LO[lo_t],
                              CH_LO[hi_t] if hi_t < T_TILES else NCH)

    nc.compile()
    return nc


def _get_program(mind: float):
    key = ("prog", round(mind, 18))
    if key not in _cache:
        _cache[key] = _build_program(mind)
    return _cache[key]


# ------------------------------------------------------------------
# entry point
# ------------------------------------------------------------------
def kernel(dwi, mask, design_matrix_inv, min_diffusivity):
    dwi = np.ascontiguousarray(np.asarray(dwi, dtype=np.float32)).reshape(V_TOT, G)
    mask = np.asarray(mask, dtype=np.float32).reshape(V_TOT)
    mind = float(np.asarray(min_diffusivity))

    nc = _get_program(mind)
    wpat = _wpat(design_matrix_inv)

    dwi_bf = dwi.astype(BF16_NP).view(np.uint16)  # [V_TOT, 64] bf16 bits
    in_maps = []
    for core in range(NCORES):
        pad = np.empty((V_PAD, G), dtype=np.uint16)
        pad[:V_CORE] = dwi_bf[core * V_CORE : (core + 1) * V_CORE]
        pad[V_CORE:] = np.float32(1.0).astype(BF16_NP).view(np.uint16)
        in_maps.append(
            {"dwi": _pack_core(pad).view(BF16_NP), "wpat": wpat}
        )

    res = None
    for attempt in range(3):
        try:
            res = bass_utils.run_bass_kernel_spmd(nc, in_maps, core_ids=list(range(NCORES)))
            break
        except Exception:
            if attempt == 2:
                raise
    _cache["last_result"] = res  # exec_time_ns etc. for the dev harness

    vmap_flat = _voxel_map().reshape(-1)  # [P*NCH*2]
    fa = np.empty(V_TOT, dtype=np.float32)
    for core in range(NCORES):
        fa_dev = np.asarray(res.results[core]["fa"]).reshape(-1)
        fa_pad = np.empty(V_PAD, dtype=np.float32)
        fa_pad[vmap_flat] = fa_dev
        fa[core * V_CORE : (core + 1) * V_CORE] = fa_pad[:V_CORE]

    fa *= mask
    return fa.reshape(NX, NY, NZ, 1)
